# revision 3
# baseline (speedup 1.0000x reference)
"""BitLinear (BitNet b1.58) Trainium2 kernel, 8-core data-parallel.

Reference computation (fp32):
    scale  = 127 / clip(max|x| over d_in, 1e-5)          (per token)
    xq     = clip(round(x*scale), -128, 127) / scale     (per-token int8 quant-dequant)
    s      = clip(mean|W|, 1e-5)
    wq     = clip(round(W/s), -1, 1) * s                 (ternary quant)
    out    = xq @ wq.T

Kernel strategy (per core, tokens sharded 4096/core, weight replicated):
    q  = round(x*scale)  are integers in [-127,127]  -> exact in bf16
    t  = clip(round(W/s),-1,1) in {-1,0,1}           -> exact in bf16
    q @ t.T accumulated in fp32 PSUM is EXACT integer arithmetic, then
    out = psum * (absmax * s / 127) per token.

    Rounding uses the fp32 magic-number trick  round(v) = (v + 1.5*2^23) - 1.5*2^23
    (round-half-even, matches jnp.round bitwise).  The +MAGIC is fused into the
    quant multiply on ACT, the transpose (PE, fp32) moves x^T to PSUM, and one
    ACT pass does (-MAGIC subtract + bf16 cast + PSUM->SBUF drain).
"""

import numpy as np

import concourse.bass as bass
import concourse.mybir as mybir
from concourse import tile, masks
from concourse.bass_utils import run_bass_kernel_spmd

F32 = mybir.dt.float32
BF16 = mybir.dt.bfloat16

N_CORES = 8
B, S, D_IN, D_OUT = 4, 8192, 1024, 1024
TOKENS = B * S                     # 32768
TOK_PER_CORE = TOKENS // N_CORES   # 4096
TILES = TOK_PER_CORE // 128        # 32
KT = D_IN // 128                   # 8 contraction k-tiles
OB = D_OUT // 128                  # 8 output row blocks of W

EPS = 1e-5
QMAX = 127.0
MAGIC = 12582912.0                     # 1.5 * 2**23 -> RNE integer rounding
THR = float(np.nextafter(np.float32(1.5), np.float32(0)))  # largest f32 < 1.5


def _split_multiwaits(nc):
    """walrus here encodes at most ONE sem wait per instruction; Tile's tail
    drain (and occasionally other insts) carry several.  Split extras into
    single-wait NOPs on the same engine, preserving order."""
    for f in nc.m.functions:
        for bb in f.blocks:
            insts = list(bb.instructions)
            if not any(
                i.sync_info and len(i.sync_info.on_wait) > 1 for i in insts
            ):
                continue
            new = []
            for ins in insts:
                si = ins.sync_info
                if si and len(si.on_wait) > 1:
                    waits = list(si.on_wait)
                    for j, w in enumerate(waits[:-1]):
                        nop = mybir.InstNoOp(
                            name=f"{ins.name}_wsp{j}", ins=[], outs=[]
                        )
                        nop.engine = ins.engine
                        nop.sync_info = mybir.SyncInfo(on_wait=[w], on_update=[])
                        new.append(nop)
                    ins.sync_info = mybir.SyncInfo(
                        on_wait=[waits[-1]], on_update=list(si.on_update)
                    )
                new.append(ins)
            bb.instructions = new


def build_program():
    nc = bass.Bass(trn_type="TRN2")
    x_d = nc.dram_tensor("x", [TOK_PER_CORE, D_IN], F32, kind="ExternalInput")
    w_d = nc.dram_tensor("weight", [D_OUT, D_IN], F32, kind="ExternalInput")
    o_d = nc.dram_tensor("out", [TOK_PER_CORE, D_OUT], F32, kind="ExternalOutput")

    Copy = mybir.ActivationFunctionType.Copy
    AX = mybir.AxisListType.X
    op = mybir.AluOpType

    with tile.TileContext(nc) as tc:
        from contextlib import ExitStack

        with ExitStack() as ctx:
            singles = ctx.enter_context(tc.tile_pool(name="singles", bufs=1))

            ident_f32 = singles.tile([128, 128], F32)
            masks.make_identity(nc, ident_f32[:])
            ident_bf16 = singles.tile([128, 128], BF16)
            masks.make_identity(nc, ident_bf16[:])
            ones_col = singles.tile([128, 1], F32)
            nc.vector.memset(ones_col[:], 1.0)
            ones_row = singles.tile([1, 128], F32)
            nc.vector.memset(ones_row[:], 1.0)
            c127 = singles.tile([128, 1], F32)
            nc.vector.memset(c127[:], QMAX)
            bc2 = singles.tile([128, 2], F32)    # [s, 1/s] broadcast to 128 parts
            s127_bc = singles.tile([128, 1], F32)  # s/127 broadcast

            # persistent transposed-ternary weight: tT[k][i, o] (bf16)
            tT = [singles.tile([128, D_OUT], BF16, name=f"tT{k}", tag=f"tT{k}") for k in range(KT)]

            # ---------------- weight phase ----------------
            with (
                tc.tile_pool(name="wpool", bufs=1) as wpool,
                tc.tile_pool(name="wtmp", bufs=2) as wtmp,
                tc.tile_pool(name="wps", bufs=1, space="PSUM") as wps,
                tc.tile_pool(name="wpst", bufs=2, space="PSUM") as wpst,
            ):
                w_t = [wpool.tile([128, D_IN], F32, name=f"w{ob}", tag=f"w{ob}") for ob in range(OB)]
                for ob in range(OB):
                    nc.sync.dma_start(w_t[ob][:], w_d[ob * 128:(ob + 1) * 128, :])

                colsum = wpool.tile([128, OB], F32)
                for ob in range(OB):
                    nc.vector.tensor_reduce(
                        colsum[:, ob:ob + 1], w_t[ob][:], axis=AX, op=op.add,
                        apply_absolute_value=True,
                    )
                colsum2 = wpool.tile([128, 1], F32)
                nc.vector.tensor_reduce(colsum2[:], colsum[:], axis=AX, op=op.add)

                ps_s = wps.tile([1, 1], F32, name="ps_s", tag="ps_s", padded_shape=[2, 1])
                nc.tensor.matmul(ps_s[:], ones_col[:], colsum2[:])
                pair = wpool.tile([1, 2], F32)
                # mean, clipped at EPS -> pair[0,0] = s
                nc.scalar.activation(pair[:, 0:1], ps_s[:], Copy, scale=1.0 / (D_OUT * D_IN))
                nc.vector.tensor_scalar_max(pair[:, 0:1], pair[:, 0:1], EPS)
                nc.vector.reciprocal(pair[:, 1:2], pair[:, 0:1])   # pair[0,1] = 1/s
                ps_bc = wps.tile([128, 2], F32, tag="ps_bc")
                nc.tensor.matmul(ps_bc[:], ones_row[:], pair[:])
                nc.scalar.copy(bc2[:], ps_bc[:])
                # s127_bc = s / 127 broadcast
                nc.vector.tensor_scalar(s127_bc[:], bc2[:, 0:1], 1.0 / QMAX, None, op0=op.mult)

                # ternary quantize (natural [o, i] layout, bf16)
                t_nat = [wpool.tile([128, D_IN], BF16, name=f"tn{ob}", tag=f"tn{ob}") for ob in range(OB)]
                for ob in range(OB):
                    y0 = wtmp.tile([128, D_IN], F32, name="y0", tag="y0")
                    nc.vector.tensor_scalar(
                        y0[:], w_t[ob][:], bc2[:, 1:2], THR, op0=op.mult, op1=op.min
                    )
                    y1 = wtmp.tile([128, D_IN], F32, name="y1", tag="y1")
                    nc.vector.tensor_scalar(
                        y1[:], y0[:], -THR, MAGIC, op0=op.max, op1=op.add
                    )
                    nc.scalar.activation(t_nat[ob][:], y1[:], Copy, bias=-MAGIC)

                # transpose to tT[k][i, o]
                for k in range(KT):
                    ps_t = wpst.tile([128, D_OUT], BF16, tag="ps_t")
                    for ob in range(OB):
                        nc.tensor.transpose(
                            ps_t[:, ob * 128:(ob + 1) * 128],
                            t_nat[ob][:, k * 128:(k + 1) * 128],
                            ident_bf16[:],
                        )
                    nc.scalar.copy(tT[k][:], ps_t[:])

            # ---------------- token stream ----------------
            xpool = ctx.enter_context(tc.tile_pool(name="xpool", bufs=3))
            xmpool = ctx.enter_context(tc.tile_pool(name="xmpool", bufs=2))
            qtpool = ctx.enter_context(tc.tile_pool(name="qtpool", bufs=3))
            outpool = ctx.enter_context(tc.tile_pool(name="outpool", bufs=3))
            smpool = ctx.enter_context(tc.tile_pool(name="smpool", bufs=4))
            psq = ctx.enter_context(tc.tile_pool(name="psq", bufs=2, space="PSUM"))
            pso = ctx.enter_context(tc.tile_pool(name="pso", bufs=2, space="PSUM"))

            qT_live = {}
            coef_live = {}

            def stage_a(n):
                x_t = xpool.tile([128, D_IN], F32, tag="x")
                nc.sync.dma_start(x_t[:], x_d[n * 128:(n + 1) * 128, :])
                am = smpool.tile([128, 1], F32, tag="am")
                nc.vector.tensor_reduce(
                    am[:], x_t[:], axis=AX, op=op.max, apply_absolute_value=True
                )
                amc = smpool.tile([128, 1], F32, tag="amc")
                nc.vector.tensor_scalar_max(amc[:], am[:], EPS)
                ram = smpool.tile([128, 1], F32, tag="ram")
                nc.vector.reciprocal(ram[:], amc[:])
                scl = smpool.tile([128, 1], F32, tag="scl")
                nc.vector.tensor_scalar(scl[:], ram[:], QMAX, None, op0=op.mult)
                coef = smpool.tile([128, 1], F32, tag="coef")
                nc.vector.tensor_scalar(
                    coef[:], amc[:], s127_bc[:], None, op0=op.mult
                )
                xm = xmpool.tile([128, D_IN], F32, tag="xm")
                nc.scalar.activation(xm[:], x_t[:], Copy, bias=MAGIC, scale=scl[:])
                ps_q = psq.tile([128, D_IN], F32, tag="ps_q")
                for k in range(KT):
                    nc.tensor.transpose(
                        ps_q[:, k * 128:(k + 1) * 128],
                        xm[:, k * 128:(k + 1) * 128],
                        ident_f32[:],
                    )
                qT = qtpool.tile([128, D_IN], BF16, tag="qT")
                nc.scalar.activation(qT[:], ps_q[:], Copy, bias=-MAGIC)
                qT_live[n] = qT
                coef_live[n] = coef

            def stage_b(n):
                qT = qT_live.pop(n)
                coef = coef_live.pop(n)
                ps_o = pso.tile([128, D_OUT], F32, tag="ps_o")
                for k in range(KT):
                    for oh in range(2):
                        nc.tensor.matmul(
                            ps_o[:, oh * 512:(oh + 1) * 512],
                            qT[:, k * 128:(k + 1) * 128],
                            tT[k][:, oh * 512:(oh + 1) * 512],
                            start=(k == 0),
                            stop=(k == KT - 1),
                        )
                out_sb = outpool.tile([128, D_OUT], F32, tag="osb")
                nc.scalar.activation(
                    out_sb[:, 0:512], ps_o[:, 0:512], Copy, scale=coef[:]
                )
                nc.vector.tensor_scalar(
                    out_sb[:, 512:1024], ps_o[:, 512:1024], coef[:], None,
                    op0=op.mult,
                )
                nc.sync.dma_start(o_d[n * 128:(n + 1) * 128, :], out_sb[:])

            stage_a(0)
            for n in range(TILES):
                if n + 1 < TILES:
                    stage_a(n + 1)
                stage_b(n)

    _split_multiwaits(nc)
    return nc


_NC_CACHE = None


def _get_nc():
    global _NC_CACHE
    if _NC_CACHE is None:
        _NC_CACHE = build_program()
    return _NC_CACHE


def kernel(x: np.ndarray, weight: np.ndarray, trace: bool = False):
    assert x.shape == (B, S, D_IN) and weight.shape == (D_OUT, D_IN)
    nc = _get_nc()
    xf = np.ascontiguousarray(x.reshape(TOKENS, D_IN), dtype=np.float32)
    w = np.ascontiguousarray(weight, dtype=np.float32)
    in_maps = [
        {
            "x": xf[c * TOK_PER_CORE:(c + 1) * TOK_PER_CORE],
            "weight": w,
        }
        for c in range(N_CORES)
    ]
    res = run_bass_kernel_spmd(nc, in_maps, core_ids=list(range(N_CORES)), trace=trace)
    kernel.last_results = res
    out = np.concatenate([res.results[c]["out"] for c in range(N_CORES)], axis=0)
    return out.reshape(B, S, D_OUT)


kernel.last_results = None
